# revision 29
# baseline (speedup 1.0000x reference)
"""GQA causal attention (B=2, S=2048, D=2048, 16 q heads / 4 kv heads, RoPE)
for 8 Trainium2 NeuronCores.

Sharding: core i = (batch b = i//4, kv-head group g = i%4). Each core computes
its group's Q/K/V projections, RoPE, causal attention and the partial output
projection; the host sums the 4 per-group partials per batch.

On-core layout is fully "transposed" (features on partitions):
  xT [D, S], QT/KT [d, S] -> QK scores land as [k, q], softmax runs along k
  (partitions) with the denominator computed by an all-ones matmul, and PV
  accumulates out^T [d, q] directly in PSUM. The final projection contracts
  over the group's 512 head-dims on partitions.

Everything is bf16 except PSUM accumulation, the exp input (fp32 scores in
PSUM) and the RoPE temporaries. The whole kernel is one fused pipeline:
per 512-token slice j we emit {K proj, RoPE(k), V proj, V transpose,
Q proj + RoPE(q) per head, attention over k-tiles 0..4(j+1), output
projection}, so the tensor engine never drains between "phases". QK scores
for two k-tiles share one 2-bank PSUM tile so exp runs 1024 wide; softmax
denominators are tree-summed on the DVE (4 tiles -> 1 ones-matmul). All
HBM operands are pre-swizzled on the host so every DMA moves 4-16KB of
contiguous bytes per partition row.
"""

import sys
import types

sys.path.insert(0, "/opt/trn_rl_repo")

# If tracing is ever requested (e.g. BASS_TRACE=1 in the environment),
# concourse needs antenv.axon_hooks, which this image lacks; provide it.
try:
    import antenv  # noqa: F401

    if "antenv.axon_hooks" not in sys.modules:
        from trn_agent_boot.trn_boot import _ntff_profile_via_ctypes

        _mod = types.ModuleType("antenv.axon_hooks")
        _hook = _ntff_profile_via_ctypes("/opt/axon/libaxon_pjrt.so")
        _mod.get_axon_ntff_profile_hook = lambda: _hook
        sys.modules["antenv.axon_hooks"] = _mod
except Exception:
    pass

import numpy as np
import ml_dtypes
from contextlib import ExitStack

import concourse.bacc as bacc
import concourse.mybir as mybir
import concourse.tile as tile
from concourse.bass_utils import run_bass_kernel_spmd

B, S, DIM = 2, 2048, 2048
N_HEADS, N_KV, HD = 16, 4, 128
HPG = N_HEADS // N_KV      # q heads per kv group
GD = HPG * HD              # 512 = group width
P = 128
NS = S // 512              # 4 s-slices of 512
NC = DIM // P              # 16 contraction chunks of 128
NKT = S // P               # 16 k tiles
F32 = mybir.dt.float32
BF16 = mybir.dt.bfloat16
BF = ml_dtypes.bfloat16
SCALE = 1.0 / float(np.sqrt(HD))

# bf16 consts column layout
C_RT = 0          # [128]  RoPE rotation (R.T)
C_ID = 128        # [128]  identity
C_ONES = 256      # [128]  all-ones
C_COS = 384       # [2048] cos, repeated x2 along d
C_BM = 2432       # [4*512] causal band masks, multiplicative 0/1
NC16 = 4480

_CACHE = {}


def _build():
    nc = bacc.Bacc()
    # All pre-swizzled on the host: per-partition rows are contiguous.
    xh = nc.dram_tensor("xh", [P, NS, NC, 512], BF16, kind="ExternalInput")
    wqh = nc.dram_tensor("wqh", [P, HPG, NC, HD], BF16, kind="ExternalInput")
    wkh = nc.dram_tensor("wkh", [P, NC, HD], BF16, kind="ExternalInput")
    wvh = nc.dram_tensor("wvh", [P, NC, HD], BF16, kind="ExternalInput")
    woh = nc.dram_tensor("woh", [P, HPG, DIM], BF16, kind="ExternalInput")
    c16 = nc.dram_tensor("c16", [P, NC16], BF16, kind="ExternalInput")
    c32 = nc.dram_tensor("c32", [P, S], F32, kind="ExternalInput")
    out = nc.dram_tensor("out", [S, DIM], BF16, kind="ExternalOutput")

    EXP = mybir.ActivationFunctionType.Exp

    with tile.TileContext(nc) as tc, ExitStack() as ctx:
        cpool = ctx.enter_context(tc.tile_pool(name="consts", bufs=1))
        persist = ctx.enter_context(tc.tile_pool(name="persist", bufs=1))
        xpool = ctx.enter_context(tc.tile_pool(name="xs", bufs=2))
        vtpool = ctx.enter_context(tc.tile_pool(name="vt", bufs=2))
        tmpp = ctx.enter_context(tc.tile_pool(name="ropetmp", bufs=4))
        ptp = ctx.enter_context(tc.tile_pool(name="pt", bufs=5))
        dsp = ctx.enter_context(tc.tile_pool(name="ds", bufs=4))
        recp = ctx.enter_context(tc.tile_pool(name="rec", bufs=2))
        outp = ctx.enter_context(tc.tile_pool(name="outp", bufs=4))
        # PSUM: psP 2x[P,512] (projection chains + softmax denominator),
        # psQ 2x[P,1024] (QK score pairs; also out-proj), psV 2x[P,512]
        # (PV accumulators; also RoPE rot + V-transpose) = 8 banks exactly.
        psP = ctx.enter_context(tc.tile_pool(name="psP", bufs=2, space="PSUM"))
        psQ = ctx.enter_context(tc.tile_pool(name="psQ", bufs=2, space="PSUM"))
        psV = ctx.enter_context(tc.tile_pool(name="psV", bufs=2, space="PSUM"))

        c16_sb = cpool.tile([P, NC16], BF16, name="c16_sb")
        sin_sb = cpool.tile([P, S], F32, name="sin_sb")
        rt = c16_sb[:, C_RT:C_RT + 128]
        ident = c16_sb[:, C_ID:C_ID + 128]
        ones = c16_sb[:, C_ONES:C_ONES + 128]
        cosf = c16_sb[:, C_COS:C_COS + S]
        bm = c16_sb[:, C_BM:C_BM + 2048]

        wq_sb = persist.tile([P, HPG, NC, HD], BF16, name="wq_sb")
        wk_sb = persist.tile([P, NC, HD], BF16, name="wk_sb")
        wv_sb = persist.tile([P, NC, HD], BF16, name="wv_sb")
        wo_sb = persist.tile([P, HPG, DIM], BF16, name="wo_sb")
        q_sb = persist.tile([P, HPG, S], BF16, name="q_sb")
        k_sb = persist.tile([P, S], BF16, name="k_sb")
        v_sb = persist.tile([P, NKT, HD], BF16, name="v_sb")
        attn_sb = persist.tile([P, HPG, S], BF16, name="attn_sb")
        warm_sb = persist.tile([P, 512], BF16, name="warm_sb")

        # ---- warmup: keep PE busy (HAM un-throttle) while first DMAs land
        nc.vector.memset(warm_sb, 0.0)
        for _ in range(16):
            wps = psQ.tile([P, 512], F32, name="warm_ps", tag="qk")
            nc.tensor.matmul(wps, warm_sb[:, :P], warm_sb, start=True, stop=True)

        # ---- startup DMAs; x chunks + weights spread over the three DMA
        # queues in projection-consumption order (K, Q0.., V), per-chunk so
        # the K chain unblocks incrementally
        xs_j = {}
        xs0 = xpool.tile([P, NC, 512], BF16, name="xs", tag="xs")
        xs_j[0] = xs0
        for cc in range(0, 6):
            nc.sync.dma_start(out=xs0[:, cc, :], in_=xh[:, 0, cc, :])
        nc.gpsimd.dma_start(out=wk_sb, in_=wkh[:, :, :])
        for cc in range(6, 11):
            nc.gpsimd.dma_start(out=xs0[:, cc, :], in_=xh[:, 0, cc, :])
        nc.scalar.dma_start(out=c16_sb[:, 0:C_COS + 512],
                            in_=c16[:, 0:C_COS + 512])
        nc.scalar.dma_start(out=sin_sb[:, 0:512], in_=c32[:, 0:512])
        for cc in range(11, 16):
            nc.scalar.dma_start(out=xs0[:, cc, :], in_=xh[:, 0, cc, :])
        nc.scalar.dma_start(out=wq_sb[:, 0], in_=wqh[:, 0])
        # remaining weights, roughly in consumption order
        nc.sync.dma_start(out=wq_sb[:, 1], in_=wqh[:, 1])
        nc.gpsimd.dma_start(out=wq_sb[:, 2], in_=wqh[:, 2])
        nc.gpsimd.dma_start(out=wq_sb[:, 3], in_=wqh[:, 3])
        nc.gpsimd.dma_start(out=wv_sb, in_=wvh[:, :, :])
        nc.scalar.dma_start(out=c16_sb[:, C_BM:C_BM + 2048],
                            in_=c16[:, C_BM:C_BM + 2048])
        nc.scalar.dma_start(out=wo_sb, in_=woh[:, :, :])

        dma_engs = [nc.sync, nc.gpsimd, nc.scalar]
        out_dma_rr = [0]

        def rope(src):
            t2 = tmpp.tile([P, 512], F32, name="t2")
            nc.vector.tensor_mul(t2, src, cosf[:, sl])
            rot = psV.tile([P, 512], F32, name="rot", tag="pv")
            nc.tensor.matmul(rot, rt, src, start=True, stop=True)
            t1 = tmpp.tile([P, 512], F32, name="t1")
            nc.vector.tensor_mul(t1, rot, sin_sb[:, sl])
            nc.vector.tensor_add(src, t1, t2)

        # slice 0's chunks land round-robin from the three DMA queues
        # (sync c0-5, gpsimd c6-10, scalar c11-15); consume them in arrival
        # order — accumulation is commutative
        RR0 = [0, 6, 11, 1, 7, 12, 2, 8, 13, 3, 9, 14, 4, 10, 15, 5]

        for j in range(NS):
            sl = slice(512 * j, 512 * (j + 1))
            xs = xs_j[j]
            corder = RR0 if j == 0 else list(range(NC))

            # prefetch x for slice j+1 (sync queue, behind slice j)
            if j + 1 < NS:
                t = xpool.tile([P, NC, 512], BF16, name="xs", tag="xs")
                nc.sync.dma_start(out=t, in_=xh[:, j + 1, :, :])
                xs_j[j + 1] = t
                nc.scalar.dma_start(
                    out=c16_sb[:, C_COS + 512 * (j + 1):C_COS + 512 * (j + 2)],
                    in_=c16[:, C_COS + 512 * (j + 1):C_COS + 512 * (j + 2)])
                nc.scalar.dma_start(
                    out=sin_sb[:, 512 * (j + 1):512 * (j + 2)],
                    in_=c32[:, 512 * (j + 1):512 * (j + 2)])

            # ---- K projection + RoPE(k) ----
            ps_k = psP.tile([P, 512], F32, name="psp", tag="pp")
            for i, c in enumerate(corder):
                nc.tensor.matmul(ps_k, wk_sb[:, c, :], xs[:, c, :],
                                 start=(i == 0), stop=(i == NC - 1))
            nc.scalar.copy(k_sb[:, sl], ps_k)
            rope(k_sb[:, sl])

            # ---- Q projections + RoPE(q), per head ----
            for t in range(HPG):
                ps_q = psP.tile([P, 512], F32, name="psp", tag="pp")
                for i, c in enumerate(corder):
                    nc.tensor.matmul(ps_q, wq_sb[:, t, c, :], xs[:, c, :],
                                     start=(i == 0), stop=(i == NC - 1))
                nc.scalar.copy(q_sb[:, t, sl], ps_q)
                rope(q_sb[:, t, sl])

            # ---- V projection + transpose (V weights arrive last) ----
            ps_v = psP.tile([P, 512], F32, name="psp", tag="pp")
            for i, c in enumerate(corder):
                nc.tensor.matmul(ps_v, wv_sb[:, c, :], xs[:, c, :],
                                 start=(i == 0), stop=(i == NC - 1))
            vt = vtpool.tile([P, 512], BF16, name="vt_sb")
            nc.scalar.copy(vt, ps_v)
            for i in range(4):
                tr = psV.tile([P, 512], BF16, name="tr", tag="pv")
                nc.tensor.transpose(tr[:, :P], vt[:, P * i:P * (i + 1)], ident)
                nc.vector.tensor_copy(v_sb[:, 4 * j + i, :], tr[:, :P])

            # ---- attention for slice j ----
            nkt = 4 * (j + 1)
            npair = nkt // 2
            for h in range(HPG):
                q_hi = q_sb[:, h, sl]
                pv = psV.tile([P, 512], F32, name="pv", tag="pv")
                den = psP.tile([P, 512], F32, name="den", tag="pp")
                # pairs of k-tiles; diagonal (masked) pairs first so their
                # longer exp+mask chains hide behind later matmuls
                pairs = ([(4 * j, 4 * j + 1, 0), (4 * j + 2, 4 * j + 3, 1)]
                         + [(2 * m, 2 * m + 1, None) for m in range(2 * j)])
                pts = [None] * npair
                dsums = [None] * npair
                sqs = []

                def score(p):
                    kt0, kt1, bi = pairs[p]
                    qk = psQ.tile([P, 1024], F32, name="qk", tag="qk")
                    nc.tensor.matmul(qk[:, 0:512], k_sb[:, P * kt0:P * (kt0 + 1)],
                                     q_hi, start=True, stop=True)
                    nc.tensor.matmul(qk[:, 512:1024], k_sb[:, P * kt1:P * (kt1 + 1)],
                                     q_hi, start=True, stop=True)
                    pt = ptp.tile([P, 1024], BF16, name="pt")
                    nc.scalar.activation(pt, qk, EXP, scale=SCALE)
                    if bi is not None:
                        nc.vector.tensor_mul(pt, pt,
                                             bm[:, 1024 * bi:1024 * (bi + 1)])
                    pts[p] = pt

                def accum(p):
                    kt0, kt1, bi = pairs[p]
                    for z, kt in enumerate((kt0, kt1)):
                        r = kt - 4 * j
                        lo = 128 * r if (bi is not None and r >= 1) else 0
                        nc.tensor.matmul(pv[:, lo:], v_sb[:, kt, :],
                                         pts[p][:, 512 * z + lo:512 * (z + 1)],
                                         start=(p == 0 and z == 0),
                                         stop=(p == npair - 1 and z == 1))
                    ds = dsp.tile([P, 512], BF16, name="ds")
                    nc.vector.tensor_add(ds, pts[p][:, 0:512], pts[p][:, 512:1024])
                    dsums[p] = ds
                    if p % 2 == 1:
                        sq = dsp.tile([P, 512], BF16, name="sq")
                        nc.vector.tensor_add(sq, dsums[p - 1], ds)
                        sqs.append(sq)

                score(0)
                for p in range(1, npair):
                    score(p)
                    accum(p - 1)
                accum(npair - 1)
                # denominator matmuls batched at head end: the den PSUM slot
                # (shared with the projection chains) is held briefly instead
                # of across the whole head
                for qi, sq in enumerate(sqs):
                    nc.tensor.matmul(den, ones, sq,
                                     start=(qi == 0), stop=(qi == len(sqs) - 1))

                rec = recp.tile([P, 512], F32, name="rec")
                nc.vector.reciprocal_approx_fast(rec, den)
                nc.vector.tensor_mul(attn_sb[:, h, sl], pv, rec)

            # ---- output projection for the 4 s-tiles of this slice ----
            for st in range(4 * j, 4 * (j + 1)):
                for e in range(NS):
                    ops = psQ.tile([P, 512], F32, name="ops", tag="qk")
                    for hc in range(HPG):
                        nc.tensor.matmul(
                            ops, attn_sb[:, hc, P * st:P * (st + 1)],
                            wo_sb[:, hc, 512 * e:512 * (e + 1)],
                            start=(hc == 0), stop=(hc == HPG - 1))
                    osb = outp.tile([P, 512], BF16, name="osb")
                    nc.vector.tensor_copy(osb, ops)
                    eng = dma_engs[out_dma_rr[0] % 3]
                    out_dma_rr[0] += 1
                    eng.dma_start(
                        out=out[P * st:P * (st + 1), 512 * e:512 * (e + 1)],
                        in_=osb)

    nc.compile()
    return nc


def _consts16(freqs_cos):
    c = np.zeros((P, NC16), np.float32)
    rtm = np.zeros((P, P), np.float32)
    idx = np.arange(0, P, 2)
    rtm[idx, idx + 1] = 1.0    # (R.T)[2j, 2j+1] = +1
    rtm[idx + 1, idx] = -1.0   # (R.T)[2j+1, 2j] = -1
    c[:, C_RT:C_RT + P] = rtm
    c[:, C_ID:C_ID + P] = np.eye(P, dtype=np.float32)
    c[:, C_ONES:C_ONES + P] = 1.0
    c[:, C_COS:C_COS + S] = np.repeat(np.asarray(freqs_cos, np.float32).T, 2,
                                      axis=0)
    ki = np.arange(P)[:, None]
    qi = np.arange(512)[None, :]
    for r in range(4):
        c[:, C_BM + 512 * r:C_BM + 512 * (r + 1)] = \
            (ki <= qi - P * r).astype(np.float32)
    return c.astype(BF)


def _swiz_w(wT, width):
    # [DIM, width] -> [P, NC, width] with [p, c, :] = wT[128c + p, :]
    return np.ascontiguousarray(
        wT.reshape(NC, P, width).transpose(1, 0, 2)).astype(BF)


def _in_maps(x, wq, wk, wv, wo, freqs_cos, freqs_sin):
    x = np.asarray(x, np.float32)
    wq = np.asarray(wq, np.float32)
    wk = np.asarray(wk, np.float32)
    wv = np.asarray(wv, np.float32)
    wo = np.asarray(wo, np.float32)
    c16a = _consts16(freqs_cos)
    c32a = np.ascontiguousarray(
        np.repeat(np.asarray(freqs_sin, np.float32).T, 2, axis=0))
    xhs = []
    for b in range(B):
        xT = x[b].T  # [DIM, S]
        # [p, j, c, s] = xT[128c + p, 512j + s]
        xhs.append(np.ascontiguousarray(
            xT.reshape(NC, P, NS, 512).transpose(1, 2, 0, 3)).astype(BF))
    maps = []
    for core in range(8):
        b, g = divmod(core, 4)
        wqT = np.ascontiguousarray(wq[GD * g:GD * (g + 1), :].T)  # [DIM, GD]
        maps.append({
            "xh": xhs[b],
            "wqh": np.ascontiguousarray(
                wqT.reshape(NC, P, HPG, HD).transpose(1, 2, 0, 3)).astype(BF),
            "wkh": _swiz_w(np.ascontiguousarray(wk[HD * g:HD * (g + 1), :].T), HD),
            "wvh": _swiz_w(np.ascontiguousarray(wv[HD * g:HD * (g + 1), :].T), HD),
            "woh": np.ascontiguousarray(
                wo[:, GD * g:GD * (g + 1)].T.reshape(HPG, P, DIM)
                .transpose(1, 0, 2)).astype(BF),
            "c16": c16a,
            "c32": c32a,
        })
    return maps


def _get_nc():
    if "nc" not in _CACHE:
        _CACHE["nc"] = _build()
    return _CACHE["nc"]


def _run(in_maps, trace=False):
    return run_bass_kernel_spmd(_get_nc(), in_maps, core_ids=list(range(8)),
                                trace=trace)


def kernel(x, wq, wk, wv, wo, freqs_cos, freqs_sin):
    res = _run(_in_maps(x, wq, wk, wv, wo, freqs_cos, freqs_sin))
    out = np.zeros((B, S, DIM), np.float32)
    for core in range(8):
        b = core // 4
        out[b] += res.results[core]["out"].astype(np.float32)
    return out
